# revision 28
# baseline (speedup 1.0000x reference)
"""Multi-head attention (B=1, S=2048, D=2048, H=16, d_k=128) on 8 Trainium2
NeuronCores via Bass/Tile.

Sharding: tensor-parallel over heads. Each core owns 2 heads: it gets the
column shards of Wq/Wk/Wv and the row shard of Wo for those heads, computes
its partial output projection, and the host sums the 8 partials (the
all-reduce equivalent) and adds biases.

All matmuls run in fp16 (single-pass, full PE rate; fp32 PSUM
accumulation). Measured end-to-end relative error ~1e-3 against the fp32
reference, dominated by fp16 rounding of x/W/Q/K.

Per-core dataflow (everything derived from x^T so contractions sit on the
partition axis):
  phase 1: Q^T = Wq_s.T @ x^T-chunks, K^T likewise, V = x @ Wv_s (natural).
           Inputs arrive as ~14 large DMAs (host pre-lays DRAM so each
           partition's slice is one contiguous run) instead of 162 small
           ones -- dma_start issue costs ~630ns each on the issuing queue.
  phase 2: per (head, q-chunk): S^T[k,q] = K^T.T @ Q^T into a 2-bank
           [128,1024] PSUM tile (two matmuls), ONE 1024-wide ACT exp ->
           P^T f16 in SBUF (wide exps amortize the 352-cycle ACT fixed
           cost), DVE accumulates P^T tiles in f16 (2x DVE mode), and two
           accumulating matmuls per group stream V-lhsT -> unnormalized
           O^T. The softmax denominator broadcast is a ones-lhsT matmul
           over the f16 accumulator (no GPSIMD -- its partition_all_reduce
           took 3.6us and stalled DVE via the shared SBUF port), then DVE
           reciprocal+multiply normalizes.
  phase 3: out_partial[q,e] = O^T.T @ Wo_s; its matmul groups are
           interleaved one-per-attention-group into phase 2 so the PE
           stays dense (no HAM re-throttle) and the tail is short.
           Output is written f16 (host sums partials in f64).
"""

import sys

sys.path.insert(0, "/opt/trn_rl_repo")

import numpy as np

S = 2048
D = 2048
H = 16
DK = 128
N_CORES = 8
HEADS_PER_CORE = H // N_CORES  # 2
DPC = HEADS_PER_CORE * DK  # 256, per-core projection width
SCALE = 1.0 / np.sqrt(DK)

TRACE = False  # test.py flips this to get an NTFF profile + exec time
_LAST_EXEC_NS = [None]
_LAST_RESULTS = [None]

_PROGRAM = [None]


def _build_program():
    from concourse import bacc, mybir
    from concourse.tile import TileContext

    f32 = mybir.dt.float32
    f16 = mybir.dt.float16

    nc = bacc.Bacc()
    # host-side layouts (contiguous per-partition runs for fat DMA descriptors):
    #   xt4[sc, p, dt*512+s] : x^T chunk sc, 16KB runs
    #   wq/wk/wv[p, dt*256+n]: 8KB runs     wo[p, t*2048+e]: 8KB runs
    #   out[qt, p, e]        : 4KB runs
    xt4 = nc.declare_dram_parameter("xt4", [4, 128, 16 * 512], f16, isOutput=False)
    wq = nc.declare_dram_parameter("wq", [128, 16 * DPC], f16, isOutput=False)
    wk = nc.declare_dram_parameter("wk", [128, 16 * DPC], f16, isOutput=False)
    wv = nc.declare_dram_parameter("wv", [128, 16 * DPC], f16, isOutput=False)
    wo = nc.declare_dram_parameter("wo", [128, 2 * S], f16, isOutput=False)
    out = nc.declare_dram_parameter("out", [16, 128, S], f16, isOutput=True)

    ND = D // 128  # 16 d-tiles of the model dim
    NS = S // 128  # 16 s-tiles
    NQ = S // 512  # 4 q/s chunks
    EXP = mybir.ActivationFunctionType.Exp
    CPY = mybir.ActivationFunctionType.Copy

    with TileContext(nc) as tc:
        with (
            tc.tile_pool(name="wpool", bufs=1) as wpool,
            tc.tile_pool(name="xpool", bufs=3) as xpool,
            tc.tile_pool(name="qkv", bufs=1) as qkv,
            tc.tile_pool(name="ppool", bufs=3) as ppool,
            tc.tile_pool(name="apool", bufs=2) as apool,
            tc.tile_pool(name="rpool", bufs=2) as rpool,
            tc.tile_pool(name="opool", bufs=8) as opool,
            tc.tile_pool(name="obpool", bufs=4) as obpool,
            tc.tile_pool(name="psA", bufs=2, space="PSUM") as psA,
            tc.tile_pool(name="psB", bufs=2, space="PSUM") as psB,
            tc.tile_pool(name="psC", bufs=2, space="PSUM") as psC,
        ):
            wq_sb = wpool.tile([128, ND * DPC], f16, tag="wq")
            wk_sb = wpool.tile([128, ND * DPC], f16, tag="wk")
            wv_sb = wpool.tile([128, ND * DPC], f16, tag="wv")
            ones_sb = wpool.tile([128, 128], f16, tag="ones")
            nc.vector.memset(ones_sb[:], 1.0)

            # per-head Q^T/K^T [128, S] and V in natural layout [128, NS*DPC]
            qt_sb = [qkv.tile([128, S], f16, tag=f"qt{h}", name=f"qt{h}") for h in range(2)]
            kt_sb = [qkv.tile([128, S], f16, tag=f"kt{h}", name=f"kt{h}") for h in range(2)]
            v_sb = qkv.tile([128, NS * DPC], f16, tag="v")

            # preload the Exp ACT table while the pipeline fills (one-time
            # ~2.7us table DMA that would otherwise stall the first real exp)
            warm = rpool.tile([128, 8], f32, tag="warm", bufs=1)
            nc.scalar.activation(warm[:], ones_sb[:, 0:8], EXP)

            # ---- input DMA issue ----
            # critical stream on the sync queue: wq/x0 interleaved at d-tile
            # granularity for the first quarter (so the first matmul starts
            # as early as possible), then coarser.  wk/wv and the bulk
            # prefetch (x1-3, wo) go on the scalar queue in parallel (idle
            # in phase 1); x3/wo block that queue until their slot retires,
            # which is long before phase 2 needs ACT.
            xts = [xpool.tile([128, ND * 512], f16, tag="xt", name=f"xt{sc}") for sc in range(NQ)]
            # chunk-0 pieces interleaved across all four tensors at 2-d-tile
            # (later 4-d-tile) granularity: the fat chunk-0 loop below
            # consumes ~200GB/s in d-tile order, matching early DMA rate.
            for pc, w in ((0, 1), (1, 1), (2, 2), (4, 2), (6, 2), (8, 4), (12, 4)):
                nc.sync.dma_start(
                    out=xts[0][:, pc * 512:(pc + w) * 512],
                    in_=xt4[0, :, pc * 512:(pc + w) * 512],
                )
                for wt, wsb in ((wq, wq_sb), (wk, wk_sb), (wv, wv_sb)):
                    nc.sync.dma_start(
                        out=wsb[:, pc * DPC:(pc + w) * DPC],
                        in_=wt[:, pc * DPC:(pc + w) * DPC],
                    )
            # bulk prefetch after the chunk-0 stream (avoids stealing its
            # bandwidth).  Everything stays on the sync queue: x3/wo block
            # their queue until their xt slot retires, and parking them on
            # the scalar queue would stall the phase-1 copies ACT now takes.
            nc.sync.dma_start(out=xts[1][:], in_=xt4[1, :, :])
            nc.sync.dma_start(out=xts[2][:], in_=xt4[2, :, :])
            nc.sync.dma_start(out=xts[3][:], in_=xt4[3, :, :])
            wo_ot = xpool.tile([128, ND * 512], f16, tag="xt", name="wo_ot")
            wo_sb = wo_ot[:, 0:2 * S]
            nc.sync.dma_start(out=wo_sb, in_=wo[:])

            # ---------------- phase 1: projections ----------------
            # chunk 0: DMA-paced interleaved accumulation -- per d-tile all
            # eight output groups advance together (Q/K both heads in the
            # two banks of a [128,1024] psC tile each, V s-tiles packed two
            # per psA bank), so each arriving 2-d-tile DMA piece is consumed
            # once and the PE runs from the first piece instead of waiting
            # for whole tensors.
            qacc = psC.tile([128, 1024], f32, tag="st", name="qacc")
            kacc = psC.tile([128, 1024], f32, tag="st", name="kacc")
            vacc = [psA.tile([128, 512], f32, tag="proj", name=f"vacc{i}")
                    for i in range(2)]
            xt0 = xts[0]
            for dt_ in range(ND):
                for acc_, w_sb in ((qacc, wq_sb), (kacc, wk_sb)):
                    for h in range(2):
                        nc.tensor.matmul(
                            acc_[:, h * 512:(h + 1) * 512],
                            w_sb[:, dt_ * DPC + h * 128: dt_ * DPC + h * 128 + 128],
                            xt0[:, dt_ * 512:(dt_ + 1) * 512],
                            start=(dt_ == 0), stop=(dt_ == ND - 1),
                        )
                for s in range(4):
                    # start=True clears has_written for the WHOLE bank, so
                    # only the first of the two groups sharing a bank may
                    # clear; the second overwrites-where-unset on its first
                    # matmul (its region's bits are clear after the even
                    # group's start).
                    nc.tensor.matmul(
                        vacc[s // 2][:, (s % 2) * 256:(s % 2) * 256 + 256],
                        xt0[:, dt_ * 512 + s * 128: dt_ * 512 + s * 128 + 128],
                        wv_sb[:, dt_ * DPC:(dt_ + 1) * DPC],
                        start=(dt_ == 0 and s % 2 == 0),
                        stop=(dt_ == ND - 1),
                        skip_group_check=True,
                    )
            # copies alternate DVE/ACT (ACT is idle in phase 1) so psA/psC
            # slots free twice as fast at chunk boundaries
            for h in range(2):
                cp0 = nc.vector.tensor_copy if h == 0 else (
                    lambda o, i: nc.scalar.activation(o, i, CPY))
                cp0(qt_sb[h][:, 0:512], qacc[:, h * 512:(h + 1) * 512])
                cp0(kt_sb[h][:, 0:512], kacc[:, h * 512:(h + 1) * 512])
            for s in range(4):
                dst_v = v_sb[:, s * DPC:(s + 1) * DPC]
                src_v = vacc[s // 2][:, (s % 2) * 256:(s % 2) * 256 + 256]
                if s % 2 == 0:
                    nc.vector.tensor_copy(dst_v, src_v)
                else:
                    nc.scalar.activation(dst_v, src_v, CPY)

            for sc in range(1, NQ):  # remaining chunks of 512 seq positions
                xt = xts[sc]
                # chunk 3 computes K^T/V first so attention can start
                # before its Q^T (only needed by the last q-chunk) is done
                wlist = ((wq_sb, qt_sb), (wk_sb, kt_sb))
                if sc == NQ - 1:
                    wlist = ((wk_sb, kt_sb), (wq_sb, qt_sb))

                def emit_v():
                    # V natural: [s_tile 128, 256] = sum_d xT[d, s_tile].T @ Wv[d, :]
                    for st in range(4):
                        s_tile = sc * 4 + st
                        ps = psA.tile([128, 512], f32, tag="proj", name="v_ps")
                        for dt_ in range(ND):
                            nc.tensor.matmul(
                                ps[:, 0:DPC],
                                xt[:, dt_ * 512 + st * 128: dt_ * 512 + st * 128 + 128],
                                wv_sb[:, dt_ * DPC:(dt_ + 1) * DPC],
                                start=(dt_ == 0),
                                stop=(dt_ == ND - 1),
                            )
                        if st % 2 == 0:
                            nc.vector.tensor_copy(
                                v_sb[:, s_tile * DPC:(s_tile + 1) * DPC], ps[:, 0:DPC]
                            )
                        else:
                            nc.scalar.activation(
                                v_sb[:, s_tile * DPC:(s_tile + 1) * DPC], ps[:, 0:DPC], CPY
                            )

                first = True
                for w_sb, dst in wlist:
                    # Q^T/K^T: [n_tile 128, s 512] = sum_d W[d, n].T @ xT[d, s]
                    for h in range(2):
                        ps = psA.tile([128, 512], f32, tag="proj", name="qk_ps")
                        for dt_ in range(ND):
                            nc.tensor.matmul(
                                ps[:],
                                w_sb[:, dt_ * DPC + h * 128: dt_ * DPC + h * 128 + 128],
                                xt[:, dt_ * 512:(dt_ + 1) * 512],
                                start=(dt_ == 0),
                                stop=(dt_ == ND - 1),
                            )
                        if h == 0:
                            nc.vector.tensor_copy(
                                dst[h][:, sc * 512:(sc + 1) * 512], ps[:]
                            )
                        else:
                            nc.scalar.activation(
                                dst[h][:, sc * 512:(sc + 1) * 512], ps[:], CPY
                            )
                    if sc == NQ - 1 and first:
                        emit_v()  # last chunk: V right after K^T
                    first = False
                if sc != NQ - 1:
                    emit_v()

            # ------- phases 2+3 interleaved -------
            # attention per (q-chunk, head); the output projection for
            # q-chunk qc is emitted one group per attention-group during the
            # next two iterations, keeping the PE dense and the tail short.
            ot_tiles = {}
            pending = []  # (qc, qt_local, ec) proj groups ready to emit
            tail_mode = [0]  # >0 once the final flush starts (copy counter)

            tail_ps = [None]

            def emit_proj_group():
                qc_, qt_, ec = pending.pop(0)
                if tail_mode[0]:
                    # final flush: attention is done, so psC's 4 banks are
                    # free -- rotate proj outputs through [128,1024] psC
                    # tiles (2 bank-halves each) for 3 groups of lookahead;
                    # psA's 2-bank rotation leaves every copy's semaphore
                    # round-trip on the critical path (~700ns/group stalls).
                    # Copies alternate strictly ACT/DVE for the same reason.
                    c = tail_mode[0] - 1
                    tail_mode[0] += 1
                    if c % 2 == 0:
                        tail_ps[0] = psC.tile([128, 1024], f32, tag="st",
                                              name="tail_ps")
                    ps = tail_ps[0][:, (c % 2) * 512:(c % 2) * 512 + 512]
                    on_act = c % 2 == 0
                else:
                    ps = psA.tile([128, 512], f32, tag="proj", name="proj_ps")[:]
                    on_act = ec == 0
                for dt_ in range(2):
                    nc.tensor.matmul(
                        ps,
                        ot_tiles[(dt_, qc_)][:, qt_ * 128:(qt_ + 1) * 128],
                        wo_sb[:, dt_ * S + ec * 512:dt_ * S + ec * 512 + 512],
                        start=(dt_ == 0),
                        stop=(dt_ == 1),
                    )
                ob = ob_tiles[(qc_, qt_)]
                if on_act:
                    nc.scalar.activation(ob[:, ec * 512:(ec + 1) * 512], ps, CPY)
                else:
                    nc.vector.tensor_copy(ob[:, ec * 512:(ec + 1) * 512], ps)
                if ec % 2 == 1:  # DMA each half as soon as its 2 copies land
                    h_ = ec // 2
                    nc.sync.dma_start(
                        out=out[qc_ * 4 + qt_, :, h_ * 1024:(h_ + 1) * 1024],
                        in_=ob[:, h_ * 1024:(h_ + 1) * 1024],
                    )

            ob_tiles = {}
            for qc in range(NQ):
                for h in range(2):
                    oT = psB.tile([128, 512], f32, tag="oT", name="oT")
                    qt_slice = qt_sb[h][:, qc * 512:(qc + 1) * 512]
                    acc = apool.tile([128, 1024], f16, tag="acc", name="acc")
                    st = psC.tile([128, 1024], f32, tag="st", name="st0")
                    for half in range(2):
                        nc.tensor.matmul(
                            st[:, half * 512:(half + 1) * 512],
                            kt_sb[h][:, half * 128:(half + 1) * 128],
                            qt_slice,
                            start=True, stop=True,
                        )
                    for g in range(8):  # 8 groups of 2 k-tiles
                        pt = ppool.tile([128, 1024], f16, tag="pt", name="pt")
                        nc.scalar.activation(pt[:], st[:], EXP, scale=float(SCALE))
                        if g + 1 < 8:
                            st = psC.tile([128, 1024], f32, tag="st", name="stn")
                            for half in range(2):
                                kt_ = (g + 1) * 2 + half
                                nc.tensor.matmul(
                                    st[:, half * 512:(half + 1) * 512],
                                    kt_sb[h][:, kt_ * 128:(kt_ + 1) * 128],
                                    qt_slice,
                                    start=True, stop=True,
                                )
                        if g == 0:
                            nc.vector.tensor_copy(acc[:], pt[:])
                        else:
                            nc.vector.tensor_add(acc[:], acc[:], pt[:])
                        for half in range(2):
                            kt_ = g * 2 + half
                            nc.tensor.matmul(
                                oT[:],
                                v_sb[:, kt_ * DPC + h * 128: kt_ * DPC + h * 128 + 128],
                                pt[:, half * 512:(half + 1) * 512],
                                start=(kt_ == 0), stop=(kt_ == NS - 1),
                            )
                        if pending:
                            emit_proj_group()
                    # softmax denominator broadcast to all partitions via a
                    # ones-lhsT matmul over the f16 accumulator
                    bc = psA.tile([128, 512], f32, tag="proj", name="bc")
                    nc.tensor.matmul(bc[:], ones_sb[:], acc[:, 0:512],
                                     start=True, stop=False)
                    nc.tensor.matmul(bc[:], ones_sb[:], acc[:, 512:1024],
                                     start=False, stop=True)
                    rc = rpool.tile([128, 512], f32, tag="recip", name="rc")
                    # ~18-bit 1/x, ~5x faster than exact reciprocal; softmax
                    # denominators are well-conditioned positives (~1e2..4e3)
                    nc.vector.reciprocal_approx_fast(rc[:], bc[:])
                    ot_tiles[(h, qc)] = opool.tile(
                        [128, 512], f16, tag="ot", name="ot_t"
                    )
                    nc.vector.tensor_mul(ot_tiles[(h, qc)][:], oT[:], rc[:])
                # both heads of qc done: queue its 16 proj groups (qt-major
                # so each out row's 4 copies finish before its DMA)
                for qt_ in range(4):
                    ob_tiles[(qc, qt_)] = obpool.tile(
                        [128, S], f16, tag="ob", name="ob"
                    )
                    for ec in range(NQ):
                        pending.append((qc, qt_, ec))
            tail_mode[0] = 1
            while pending:
                emit_proj_group()

    nc.compile()
    return nc


def _numpy_fallback(x, mask, Wq, bq, Wk, bk, Wv, bv, Wo, bo):
    B, S_, D_ = x.shape
    xf = x.reshape(S_, D_).astype(np.float64)

    def proj(W, b):
        y = xf @ W.astype(np.float64) + b.astype(np.float64)
        return y.reshape(S_, H, DK).transpose(1, 0, 2)

    Q = proj(Wq, bq)
    K = proj(Wk, bk)
    V = proj(Wv, bv)
    m = np.broadcast_to(mask, (B, H, S_, S_))
    out = np.empty((H, S_, DK))
    for h in range(H):
        sc = Q[h] @ K[h].T / np.sqrt(DK)
        sc = np.where(m[0, h], sc, -np.inf)
        sc -= sc.max(axis=-1, keepdims=True)
        e = np.exp(sc)
        p = e / e.sum(axis=-1, keepdims=True)
        out[h] = p @ V[h]
    o = out.transpose(1, 0, 2).reshape(S_, D_)
    res = o @ Wo.astype(np.float64) + bo.astype(np.float64)
    return res.reshape(B, S_, D_).astype(np.float32)


def kernel(x, mask, Wq, bq, Wk, bk, Wv, bv, Wo, bo):
    x = np.asarray(x, dtype=np.float32)
    mask = np.asarray(mask)
    Wq = np.asarray(Wq, dtype=np.float32)
    Wk = np.asarray(Wk, dtype=np.float32)
    Wv = np.asarray(Wv, dtype=np.float32)
    Wo = np.asarray(Wo, dtype=np.float32)
    bq = np.asarray(bq, dtype=np.float32)
    bk = np.asarray(bk, dtype=np.float32)
    bv = np.asarray(bv, dtype=np.float32)
    bo = np.asarray(bo, dtype=np.float32)

    # Off-benchmark shapes/masks/biases: exact numpy fallback.
    # (bk shifts every score row by a constant -> softmax-invariant; bv and bo
    # are affine in the output and folded in on the host; only bq actually
    # changes the attention pattern in a way the device kernel doesn't model.)
    if x.shape != (1, S, D) or not bool(mask.all()) or np.any(bq):
        return _numpy_fallback(x, mask, Wq, bq, Wk, bk, Wv, bv, Wo, bo)

    from concourse.bass_utils import run_bass_kernel_spmd

    if _PROGRAM[0] is None:
        _PROGRAM[0] = _build_program()
    nc = _PROGRAM[0]

    # xT[dt*128+p, sc*512+s] -> [sc, p, dt, s] (16KB contiguous per partition)
    xT = x.reshape(S, D).T.astype(np.float16)
    xt4 = np.ascontiguousarray(
        xT.reshape(16, 128, 4, 512).transpose(2, 1, 0, 3)
    ).reshape(4, 128, 16 * 512)

    def wlay(Wcol):  # [dt*128+p, n] -> [p, dt*n]
        return np.ascontiguousarray(
            Wcol.reshape(16, 128, DPC).transpose(1, 0, 2)
        ).reshape(128, 16 * DPC)

    in_maps = []
    for c in range(N_CORES):
        lo, hi = c * DPC, (c + 1) * DPC
        wo_c = np.ascontiguousarray(
            Wo[lo:hi, :].astype(np.float16).reshape(2, 128, S).transpose(1, 0, 2)
        ).reshape(128, 2 * S)
        in_maps.append(
            {
                "xt4": xt4,
                "wq": wlay(Wq[:, lo:hi].astype(np.float16)),
                "wk": wlay(Wk[:, lo:hi].astype(np.float16)),
                "wv": wlay(Wv[:, lo:hi].astype(np.float16)),
                "wo": wo_c,
            }
        )

    res = run_bass_kernel_spmd(nc, in_maps, list(range(N_CORES)), trace=TRACE)
    _LAST_EXEC_NS[0] = res.exec_time_ns
    _LAST_RESULTS[0] = res

    acc = res.results[0]["out"].astype(np.float64)
    for c in range(1, N_CORES):
        acc += res.results[c]["out"]
    acc = acc.reshape(S, D)
    # bv contributes (attn rows sum to 1) a constant bv @ Wo; bo is additive.
    acc += (bv.astype(np.float64) @ Wo) + bo
    return acc.astype(np.float32).reshape(1, S, D)


# revision 29
# speedup vs baseline: 1.1926x; 1.1926x over previous
"""Multi-head attention (B=1, S=2048, D=2048, H=16, d_k=128) on 8 Trainium2
NeuronCores via Bass/Tile.

Sharding: tensor-parallel over heads. Each core owns 2 heads: it gets the
column shards of Wq/Wk/Wv and the row shard of Wo for those heads, computes
its partial output projection, and the host sums the 8 partials (the
all-reduce equivalent) and adds biases.

All matmuls run in fp16 (single-pass, full PE rate; fp32 PSUM
accumulation). Measured end-to-end relative error ~1e-3 against the fp32
reference, dominated by fp16 rounding of x/W/Q/K.

Per-core dataflow (everything derived from x^T so contractions sit on the
partition axis):
  phase 1: Q^T = Wq_s.T @ x^T-chunks, K^T likewise, V = x @ Wv_s (natural).
           Inputs arrive as ~14 large DMAs (host pre-lays DRAM so each
           partition's slice is one contiguous run) instead of 162 small
           ones -- dma_start issue costs ~630ns each on the issuing queue.
  phase 2: per (head, q-chunk): S^T[k,q] = K^T.T @ Q^T into a 2-bank
           [128,1024] PSUM tile (two matmuls), ONE 1024-wide ACT exp ->
           P^T f16 in SBUF (wide exps amortize the 352-cycle ACT fixed
           cost), DVE accumulates P^T tiles in f16 (2x DVE mode), and two
           accumulating matmuls per group stream V-lhsT -> unnormalized
           O^T. The softmax denominator broadcast is a ones-lhsT matmul
           over the f16 accumulator (no GPSIMD -- its partition_all_reduce
           took 3.6us and stalled DVE via the shared SBUF port), then DVE
           reciprocal+multiply normalizes.
  phase 3: out_partial[q,e] = O^T.T @ Wo_s; its matmul groups are
           interleaved one-per-attention-group into phase 2 so the PE
           stays dense (no HAM re-throttle) and the tail is short.
           Output is written f16 (host sums partials in f64).
"""

import sys

sys.path.insert(0, "/opt/trn_rl_repo")

import numpy as np

S = 2048
D = 2048
H = 16
DK = 128
N_CORES = 8
HEADS_PER_CORE = H // N_CORES  # 2
DPC = HEADS_PER_CORE * DK  # 256, per-core projection width
SCALE = 1.0 / np.sqrt(DK)

TRACE = False  # test.py flips this to get an NTFF profile + exec time
_LAST_EXEC_NS = [None]
_LAST_RESULTS = [None]

_PROGRAM = [None]


def _build_program():
    from concourse import bacc, mybir
    from concourse.tile import TileContext

    f32 = mybir.dt.float32
    f16 = mybir.dt.float16

    nc = bacc.Bacc()
    # host-side layouts (contiguous per-partition runs for fat DMA descriptors):
    #   xt4[sc, p, dt*512+s] : x^T chunk sc, 16KB runs
    #   wq/wk/wv[p, dt*256+n]: 8KB runs     wo[p, t*2048+e]: 8KB runs
    #   out[qt, p, e]        : 4KB runs
    xt4 = nc.declare_dram_parameter("xt4", [4, 128, 16 * 512], f16, isOutput=False)
    wq = nc.declare_dram_parameter("wq", [128, 16 * DPC], f16, isOutput=False)
    wk = nc.declare_dram_parameter("wk", [128, 16 * DPC], f16, isOutput=False)
    wv = nc.declare_dram_parameter("wv", [128, 16 * DPC], f16, isOutput=False)
    wo = nc.declare_dram_parameter("wo", [128, 2 * S], f16, isOutput=False)
    out = nc.declare_dram_parameter("out", [16, 128, S], f16, isOutput=True)

    ND = D // 128  # 16 d-tiles of the model dim
    NS = S // 128  # 16 s-tiles
    NQ = S // 512  # 4 q/s chunks
    EXP = mybir.ActivationFunctionType.Exp
    CPY = mybir.ActivationFunctionType.Copy

    with TileContext(nc) as tc:
        with (
            tc.tile_pool(name="wpool", bufs=1) as wpool,
            tc.tile_pool(name="xpool", bufs=3) as xpool,
            tc.tile_pool(name="qkv", bufs=1) as qkv,
            tc.tile_pool(name="ppool", bufs=3) as ppool,
            tc.tile_pool(name="apool", bufs=2) as apool,
            tc.tile_pool(name="rpool", bufs=2) as rpool,
            tc.tile_pool(name="opool", bufs=8) as opool,
            tc.tile_pool(name="obpool", bufs=8) as obpool,
            tc.tile_pool(name="psA", bufs=2, space="PSUM") as psA,
            tc.tile_pool(name="psB", bufs=2, space="PSUM") as psB,
            tc.tile_pool(name="psC", bufs=2, space="PSUM") as psC,
        ):
            wq_sb = wpool.tile([128, ND * DPC], f16, tag="wq")
            wk_sb = wpool.tile([128, ND * DPC], f16, tag="wk")
            wv_sb = wpool.tile([128, ND * DPC], f16, tag="wv")
            ones_sb = wpool.tile([128, 128], f16, tag="ones")
            nc.vector.memset(ones_sb[:], 1.0)

            # per-head Q^T/K^T [128, S] and V in natural layout [128, NS*DPC]
            qt_sb = [qkv.tile([128, S], f16, tag=f"qt{h}", name=f"qt{h}") for h in range(2)]
            kt_sb = [qkv.tile([128, S], f16, tag=f"kt{h}", name=f"kt{h}") for h in range(2)]
            v_sb = qkv.tile([128, NS * DPC], f16, tag="v")

            # preload the Exp ACT table while the pipeline fills (one-time
            # ~2.7us table DMA that would otherwise stall the first real exp)
            warm = rpool.tile([128, 8], f32, tag="warm", bufs=1)
            nc.scalar.activation(warm[:], ones_sb[:, 0:8], EXP)

            # ---- input DMA issue ----
            # critical stream on the sync queue: wq/x0 interleaved at d-tile
            # granularity for the first quarter (so the first matmul starts
            # as early as possible), then coarser.  wk/wv and the bulk
            # prefetch (x1-3, wo) go on the scalar queue in parallel (idle
            # in phase 1); x3/wo block that queue until their slot retires,
            # which is long before phase 2 needs ACT.
            xts = [xpool.tile([128, ND * 512], f16, tag="xt", name=f"xt{sc}") for sc in range(NQ)]
            # chunk-0 pieces interleaved across all four tensors at 2-d-tile
            # (later 4-d-tile) granularity: the fat chunk-0 loop below
            # consumes ~200GB/s in d-tile order, matching early DMA rate.
            for pc, w in ((0, 1), (1, 1), (2, 2), (4, 2), (6, 2), (8, 4), (12, 4)):
                nc.sync.dma_start(
                    out=xts[0][:, pc * 512:(pc + w) * 512],
                    in_=xt4[0, :, pc * 512:(pc + w) * 512],
                )
                for wt, wsb in ((wq, wq_sb), (wk, wk_sb), (wv, wv_sb)):
                    nc.sync.dma_start(
                        out=wsb[:, pc * DPC:(pc + w) * DPC],
                        in_=wt[:, pc * DPC:(pc + w) * DPC],
                    )
            # bulk prefetch after the chunk-0 stream (avoids stealing its
            # bandwidth).  Everything stays on the sync queue: x3/wo block
            # their queue until their xt slot retires, and parking them on
            # the scalar queue would stall the phase-1 copies ACT now takes.
            nc.sync.dma_start(out=xts[1][:], in_=xt4[1, :, :])
            nc.sync.dma_start(out=xts[2][:], in_=xt4[2, :, :])
            nc.sync.dma_start(out=xts[3][:], in_=xt4[3, :, :])
            wo_ot = xpool.tile([128, ND * 512], f16, tag="xt", name="wo_ot")
            wo_sb = wo_ot[:, 0:2 * S]
            nc.sync.dma_start(out=wo_sb, in_=wo[:])

            # ---------------- phase 1: projections ----------------
            # chunk 0: DMA-paced interleaved accumulation -- per d-tile all
            # eight output groups advance together (Q/K both heads in the
            # two banks of a [128,1024] psC tile each, V s-tiles packed two
            # per psA bank), so each arriving 2-d-tile DMA piece is consumed
            # once and the PE runs from the first piece instead of waiting
            # for whole tensors.
            qacc = psC.tile([128, 1024], f32, tag="st", name="qacc")
            kacc = psC.tile([128, 1024], f32, tag="st", name="kacc")
            vacc = [psA.tile([128, 512], f32, tag="proj", name=f"vacc{i}")
                    for i in range(2)]
            xt0 = xts[0]
            for dt_ in range(ND):
                for acc_, w_sb in ((qacc, wq_sb), (kacc, wk_sb)):
                    for h in range(2):
                        nc.tensor.matmul(
                            acc_[:, h * 512:(h + 1) * 512],
                            w_sb[:, dt_ * DPC + h * 128: dt_ * DPC + h * 128 + 128],
                            xt0[:, dt_ * 512:(dt_ + 1) * 512],
                            start=(dt_ == 0), stop=(dt_ == ND - 1),
                        )
                for s in range(4):
                    # start=True clears has_written for the WHOLE bank, so
                    # only the first of the two groups sharing a bank may
                    # clear; the second overwrites-where-unset on its first
                    # matmul (its region's bits are clear after the even
                    # group's start).
                    nc.tensor.matmul(
                        vacc[s // 2][:, (s % 2) * 256:(s % 2) * 256 + 256],
                        xt0[:, dt_ * 512 + s * 128: dt_ * 512 + s * 128 + 128],
                        wv_sb[:, dt_ * DPC:(dt_ + 1) * DPC],
                        start=(dt_ == 0 and s % 2 == 0),
                        stop=(dt_ == ND - 1),
                        skip_group_check=True,
                    )
            # copies alternate DVE/ACT (ACT is idle in phase 1) so psA/psC
            # slots free twice as fast at chunk boundaries
            for h in range(2):
                cp0 = nc.vector.tensor_copy if h == 0 else (
                    lambda o, i: nc.scalar.activation(o, i, CPY))
                cp0(qt_sb[h][:, 0:512], qacc[:, h * 512:(h + 1) * 512])
                cp0(kt_sb[h][:, 0:512], kacc[:, h * 512:(h + 1) * 512])
            for s in range(4):
                dst_v = v_sb[:, s * DPC:(s + 1) * DPC]
                src_v = vacc[s // 2][:, (s % 2) * 256:(s % 2) * 256 + 256]
                if s % 2 == 0:
                    nc.vector.tensor_copy(dst_v, src_v)
                else:
                    nc.scalar.activation(dst_v, src_v, CPY)

            for sc in range(1, NQ):  # remaining chunks of 512 seq positions
                xt = xts[sc]
                # chunk 3 computes K^T/V first so attention can start
                # before its Q^T (only needed by the last q-chunk) is done
                wlist = ((wq_sb, qt_sb), (wk_sb, kt_sb))
                if sc == NQ - 1:
                    wlist = ((wk_sb, kt_sb), (wq_sb, qt_sb))

                def emit_v():
                    # V natural: [s_tile 128, 256] = sum_d xT[d, s_tile].T @ Wv[d, :]
                    for st in range(4):
                        s_tile = sc * 4 + st
                        ps = psA.tile([128, 512], f32, tag="proj", name="v_ps")
                        for dt_ in range(ND):
                            nc.tensor.matmul(
                                ps[:, 0:DPC],
                                xt[:, dt_ * 512 + st * 128: dt_ * 512 + st * 128 + 128],
                                wv_sb[:, dt_ * DPC:(dt_ + 1) * DPC],
                                start=(dt_ == 0),
                                stop=(dt_ == ND - 1),
                            )
                        if st % 2 == 0:
                            nc.vector.tensor_copy(
                                v_sb[:, s_tile * DPC:(s_tile + 1) * DPC], ps[:, 0:DPC]
                            )
                        else:
                            nc.scalar.activation(
                                v_sb[:, s_tile * DPC:(s_tile + 1) * DPC], ps[:, 0:DPC], CPY
                            )

                first = True
                for w_sb, dst in wlist:
                    # Q^T/K^T: [n_tile 128, s 512] = sum_d W[d, n].T @ xT[d, s]
                    for h in range(2):
                        ps = psA.tile([128, 512], f32, tag="proj", name="qk_ps")
                        for dt_ in range(ND):
                            nc.tensor.matmul(
                                ps[:],
                                w_sb[:, dt_ * DPC + h * 128: dt_ * DPC + h * 128 + 128],
                                xt[:, dt_ * 512:(dt_ + 1) * 512],
                                start=(dt_ == 0),
                                stop=(dt_ == ND - 1),
                            )
                        if h == 0:
                            nc.vector.tensor_copy(
                                dst[h][:, sc * 512:(sc + 1) * 512], ps[:]
                            )
                        else:
                            nc.scalar.activation(
                                dst[h][:, sc * 512:(sc + 1) * 512], ps[:], CPY
                            )
                    if sc == NQ - 1 and first:
                        emit_v()  # last chunk: V right after K^T
                    first = False
                if sc != NQ - 1:
                    emit_v()

            # ------- phases 2+3 interleaved -------
            # attention per (q-chunk, head); the output projection for
            # q-chunk qc is emitted one group per attention-group during the
            # next two iterations, keeping the PE dense and the tail short.
            ot_tiles = {}
            pending = []  # (qc, qt_local, ec) proj groups ready to emit
            tail_mode = [0]  # >0 once the final flush starts (copy counter)

            tail_ps = [None]

            def emit_proj_group():
                qc_, qt_, ec = pending.pop(0)
                if tail_mode[0]:
                    # final flush: attention is done, so psC's 4 banks are
                    # free -- rotate proj outputs through [128,1024] psC
                    # tiles (2 bank-halves each) for 3 groups of lookahead;
                    # psA's 2-bank rotation leaves every copy's semaphore
                    # round-trip on the critical path (~700ns/group stalls).
                    # Copies alternate strictly ACT/DVE for the same reason.
                    c = tail_mode[0] - 1
                    tail_mode[0] += 1
                    if c % 2 == 0:
                        tail_ps[0] = psC.tile([128, 1024], f32, tag="st",
                                              name="tail_ps")
                    ps = tail_ps[0][:, (c % 2) * 512:(c % 2) * 512 + 512]
                    on_act = c % 2 == 0
                else:
                    ps = psA.tile([128, 512], f32, tag="proj", name="proj_ps")[:]
                    on_act = ec == 0
                for dt_ in range(2):
                    nc.tensor.matmul(
                        ps,
                        ot_tiles[(dt_, qc_)][:, qt_ * 128:(qt_ + 1) * 128],
                        wo_sb[:, dt_ * S + ec * 512:dt_ * S + ec * 512 + 512],
                        start=(dt_ == 0),
                        stop=(dt_ == 1),
                    )
                ob = ob_tiles[(qc_, qt_)]
                if on_act:
                    nc.scalar.activation(ob[:, ec * 512:(ec + 1) * 512], ps, CPY)
                else:
                    nc.vector.tensor_copy(ob[:, ec * 512:(ec + 1) * 512], ps)
                if ec % 2 == 1:  # DMA each half as soon as its 2 copies land
                    h_ = ec // 2
                    nc.sync.dma_start(
                        out=out[qc_ * 4 + qt_, :, h_ * 1024:(h_ + 1) * 1024],
                        in_=ob[:, h_ * 1024:(h_ + 1) * 1024],
                    )

            ob_tiles = {}
            for qc in range(NQ):
                for h in range(2):
                    oT = psB.tile([128, 512], f32, tag="oT", name="oT")
                    qt_slice = qt_sb[h][:, qc * 512:(qc + 1) * 512]
                    acc = apool.tile([128, 1024], f16, tag="acc", name="acc")
                    st = psC.tile([128, 1024], f32, tag="st", name="st0")
                    for half in range(2):
                        nc.tensor.matmul(
                            st[:, half * 512:(half + 1) * 512],
                            kt_sb[h][:, half * 128:(half + 1) * 128],
                            qt_slice,
                            start=True, stop=True,
                        )
                    for g in range(8):  # 8 groups of 2 k-tiles
                        pt = ppool.tile([128, 1024], f16, tag="pt", name="pt")
                        nc.scalar.activation(pt[:], st[:], EXP, scale=float(SCALE))
                        if g + 1 < 8:
                            st = psC.tile([128, 1024], f32, tag="st", name="stn")
                            for half in range(2):
                                kt_ = (g + 1) * 2 + half
                                nc.tensor.matmul(
                                    st[:, half * 512:(half + 1) * 512],
                                    kt_sb[h][:, kt_ * 128:(kt_ + 1) * 128],
                                    qt_slice,
                                    start=True, stop=True,
                                )
                        if g == 0:
                            nc.vector.tensor_copy(acc[:], pt[:])
                        else:
                            nc.vector.tensor_add(acc[:], acc[:], pt[:])
                        for half in range(2):
                            kt_ = g * 2 + half
                            nc.tensor.matmul(
                                oT[:],
                                v_sb[:, kt_ * DPC + h * 128: kt_ * DPC + h * 128 + 128],
                                pt[:, half * 512:(half + 1) * 512],
                                start=(kt_ == 0), stop=(kt_ == NS - 1),
                            )
                        if pending:
                            emit_proj_group()
                    # softmax denominator broadcast to all partitions via a
                    # ones-lhsT matmul over the f16 accumulator
                    bc = psA.tile([128, 512], f32, tag="proj", name="bc")
                    nc.tensor.matmul(bc[:], ones_sb[:], acc[:, 0:512],
                                     start=True, stop=False)
                    nc.tensor.matmul(bc[:], ones_sb[:], acc[:, 512:1024],
                                     start=False, stop=True)
                    rc = rpool.tile([128, 512], f32, tag="recip", name="rc")
                    # ~18-bit 1/x, ~5x faster than exact reciprocal; softmax
                    # denominators are well-conditioned positives (~1e2..4e3)
                    nc.vector.reciprocal_approx_fast(rc[:], bc[:])
                    ot_tiles[(h, qc)] = opool.tile(
                        [128, 512], f16, tag="ot", name="ot_t"
                    )
                    nc.vector.tensor_mul(ot_tiles[(h, qc)][:], oT[:], rc[:])
                # both heads of qc done: queue its 16 proj groups (qt-major
                # so each out row's 4 copies finish before its DMA)
                for qt_ in range(4):
                    ob_tiles[(qc, qt_)] = obpool.tile(
                        [128, S], f16, tag="ob", name="ob"
                    )
                    for ec in range(NQ):
                        pending.append((qc, qt_, ec))
            tail_mode[0] = 1
            while pending:
                emit_proj_group()

    nc.compile()
    return nc


def _numpy_fallback(x, mask, Wq, bq, Wk, bk, Wv, bv, Wo, bo):
    B, S_, D_ = x.shape
    xf = x.reshape(S_, D_).astype(np.float64)

    def proj(W, b):
        y = xf @ W.astype(np.float64) + b.astype(np.float64)
        return y.reshape(S_, H, DK).transpose(1, 0, 2)

    Q = proj(Wq, bq)
    K = proj(Wk, bk)
    V = proj(Wv, bv)
    m = np.broadcast_to(mask, (B, H, S_, S_))
    out = np.empty((H, S_, DK))
    for h in range(H):
        sc = Q[h] @ K[h].T / np.sqrt(DK)
        sc = np.where(m[0, h], sc, -np.inf)
        sc -= sc.max(axis=-1, keepdims=True)
        e = np.exp(sc)
        p = e / e.sum(axis=-1, keepdims=True)
        out[h] = p @ V[h]
    o = out.transpose(1, 0, 2).reshape(S_, D_)
    res = o @ Wo.astype(np.float64) + bo.astype(np.float64)
    return res.reshape(B, S_, D_).astype(np.float32)


def kernel(x, mask, Wq, bq, Wk, bk, Wv, bv, Wo, bo):
    x = np.asarray(x, dtype=np.float32)
    mask = np.asarray(mask)
    Wq = np.asarray(Wq, dtype=np.float32)
    Wk = np.asarray(Wk, dtype=np.float32)
    Wv = np.asarray(Wv, dtype=np.float32)
    Wo = np.asarray(Wo, dtype=np.float32)
    bq = np.asarray(bq, dtype=np.float32)
    bk = np.asarray(bk, dtype=np.float32)
    bv = np.asarray(bv, dtype=np.float32)
    bo = np.asarray(bo, dtype=np.float32)

    # Off-benchmark shapes/masks/biases: exact numpy fallback.
    # (bk shifts every score row by a constant -> softmax-invariant; bv and bo
    # are affine in the output and folded in on the host; only bq actually
    # changes the attention pattern in a way the device kernel doesn't model.)
    if x.shape != (1, S, D) or not bool(mask.all()) or np.any(bq):
        return _numpy_fallback(x, mask, Wq, bq, Wk, bk, Wv, bv, Wo, bo)

    from concourse.bass_utils import run_bass_kernel_spmd

    if _PROGRAM[0] is None:
        _PROGRAM[0] = _build_program()
    nc = _PROGRAM[0]

    # xT[dt*128+p, sc*512+s] -> [sc, p, dt, s] (16KB contiguous per partition)
    xT = x.reshape(S, D).T.astype(np.float16)
    xt4 = np.ascontiguousarray(
        xT.reshape(16, 128, 4, 512).transpose(2, 1, 0, 3)
    ).reshape(4, 128, 16 * 512)

    def wlay(Wcol):  # [dt*128+p, n] -> [p, dt*n]
        return np.ascontiguousarray(
            Wcol.reshape(16, 128, DPC).transpose(1, 0, 2)
        ).reshape(128, 16 * DPC)

    in_maps = []
    for c in range(N_CORES):
        lo, hi = c * DPC, (c + 1) * DPC
        wo_c = np.ascontiguousarray(
            Wo[lo:hi, :].astype(np.float16).reshape(2, 128, S).transpose(1, 0, 2)
        ).reshape(128, 2 * S)
        in_maps.append(
            {
                "xt4": xt4,
                "wq": wlay(Wq[:, lo:hi].astype(np.float16)),
                "wk": wlay(Wk[:, lo:hi].astype(np.float16)),
                "wv": wlay(Wv[:, lo:hi].astype(np.float16)),
                "wo": wo_c,
            }
        )

    res = run_bass_kernel_spmd(nc, in_maps, list(range(N_CORES)), trace=TRACE)
    _LAST_EXEC_NS[0] = res.exec_time_ns
    _LAST_RESULTS[0] = res

    acc = res.results[0]["out"].astype(np.float64)
    for c in range(1, N_CORES):
        acc += res.results[c]["out"]
    acc = acc.reshape(S, D)
    # bv contributes (attn rows sum to 1) a constant bv @ Wo; bo is additive.
    acc += (bv.astype(np.float64) @ Wo) + bo
    return acc.astype(np.float32).reshape(1, S, D)


# revision 30
# speedup vs baseline: 1.2240x; 1.0263x over previous
"""Multi-head attention (B=1, S=2048, D=2048, H=16, d_k=128) on 8 Trainium2
NeuronCores via Bass/Tile.

Sharding: tensor-parallel over heads. Each core owns 2 heads: it gets the
column shards of Wq/Wk/Wv and the row shard of Wo for those heads, computes
its partial output projection, and the host sums the 8 partials (the
all-reduce equivalent) and adds biases.

All matmuls run in fp16 (single-pass, full PE rate; fp32 PSUM
accumulation). Measured end-to-end relative error ~1e-3 against the fp32
reference, dominated by fp16 rounding of x/W/Q/K.

Per-core dataflow (everything derived from x^T so contractions sit on the
partition axis):
  phase 1: Q^T = Wq_s.T @ x^T-chunks, K^T likewise, V = x @ Wv_s (natural).
           Inputs arrive as ~14 large DMAs (host pre-lays DRAM so each
           partition's slice is one contiguous run) instead of 162 small
           ones -- dma_start issue costs ~630ns each on the issuing queue.
  phase 2: per (head, q-chunk): S^T[k,q] = K^T.T @ Q^T into a 2-bank
           [128,1024] PSUM tile (two matmuls), ONE 1024-wide ACT exp ->
           P^T f16 in SBUF (wide exps amortize the 352-cycle ACT fixed
           cost), DVE accumulates P^T tiles in f16 (2x DVE mode), and two
           accumulating matmuls per group stream V-lhsT -> unnormalized
           O^T. The softmax denominator broadcast is a ones-lhsT matmul
           over the f16 accumulator (no GPSIMD -- its partition_all_reduce
           took 3.6us and stalled DVE via the shared SBUF port), then DVE
           reciprocal+multiply normalizes.
  phase 3: out_partial[q,e] = O^T.T @ Wo_s; its matmul groups are
           interleaved one-per-attention-group into phase 2 so the PE
           stays dense (no HAM re-throttle) and the tail is short.
           Output is written f16 (host sums partials in f64).
"""

import sys

sys.path.insert(0, "/opt/trn_rl_repo")

import numpy as np

S = 2048
D = 2048
H = 16
DK = 128
N_CORES = 8
HEADS_PER_CORE = H // N_CORES  # 2
DPC = HEADS_PER_CORE * DK  # 256, per-core projection width
SCALE = 1.0 / np.sqrt(DK)

TRACE = False  # test.py flips this to get an NTFF profile + exec time
_LAST_EXEC_NS = [None]
_LAST_RESULTS = [None]

_PROGRAM = [None]


def _build_program():
    from concourse import bacc, mybir
    from concourse.tile import TileContext

    f32 = mybir.dt.float32
    f16 = mybir.dt.float16

    nc = bacc.Bacc()
    # host-side layouts (contiguous per-partition runs for fat DMA descriptors):
    #   xt4[sc, p, dt*512+s] : x^T chunk sc, 16KB runs
    #   wq/wk/wv[p, dt*256+n]: 8KB runs     wo[p, t*2048+e]: 8KB runs
    #   out[qt, p, e]        : 4KB runs
    xt4 = nc.declare_dram_parameter("xt4", [4, 128, 16 * 512], f16, isOutput=False)
    wq = nc.declare_dram_parameter("wq", [128, 16 * DPC], f16, isOutput=False)
    wk = nc.declare_dram_parameter("wk", [128, 16 * DPC], f16, isOutput=False)
    wv = nc.declare_dram_parameter("wv", [128, 16 * DPC], f16, isOutput=False)
    wo = nc.declare_dram_parameter("wo", [128, 2 * S], f16, isOutput=False)
    out = nc.declare_dram_parameter("out", [16, 128, S], f16, isOutput=True)

    ND = D // 128  # 16 d-tiles of the model dim
    NS = S // 128  # 16 s-tiles
    NQ = S // 512  # 4 q/s chunks
    EXP = mybir.ActivationFunctionType.Exp
    CPY = mybir.ActivationFunctionType.Copy

    with TileContext(nc) as tc:
        with (
            tc.tile_pool(name="wpool", bufs=1) as wpool,
            tc.tile_pool(name="xpool", bufs=3) as xpool,
            tc.tile_pool(name="qkv", bufs=1) as qkv,
            tc.tile_pool(name="ppool", bufs=3) as ppool,
            tc.tile_pool(name="apool", bufs=2) as apool,
            tc.tile_pool(name="rpool", bufs=2) as rpool,
            tc.tile_pool(name="opool", bufs=8) as opool,
            tc.tile_pool(name="obpool", bufs=8) as obpool,
            tc.tile_pool(name="psA", bufs=2, space="PSUM") as psA,
            tc.tile_pool(name="psB", bufs=2, space="PSUM") as psB,
            tc.tile_pool(name="psC", bufs=2, space="PSUM") as psC,
        ):
            wq_sb = wpool.tile([128, ND * DPC], f16, tag="wq")
            wk_sb = wpool.tile([128, ND * DPC], f16, tag="wk")
            wv_sb = wpool.tile([128, ND * DPC], f16, tag="wv")
            ones_sb = wpool.tile([128, 128], f16, tag="ones")
            nc.vector.memset(ones_sb[:], 1.0)

            # per-head Q^T/K^T [128, S] and V in natural layout [128, NS*DPC]
            qt_sb = [qkv.tile([128, S], f16, tag=f"qt{h}", name=f"qt{h}") for h in range(2)]
            kt_sb = [qkv.tile([128, S], f16, tag=f"kt{h}", name=f"kt{h}") for h in range(2)]
            v_sb = qkv.tile([128, NS * DPC], f16, tag="v")

            # preload the Exp ACT table while the pipeline fills (one-time
            # ~2.7us table DMA that would otherwise stall the first real exp)
            warm = rpool.tile([128, 8], f32, tag="warm", bufs=1)
            nc.scalar.activation(warm[:], ones_sb[:, 0:8], EXP)

            # ---- input DMA issue ----
            # critical stream on the sync queue: wq/x0 interleaved at d-tile
            # granularity for the first quarter (so the first matmul starts
            # as early as possible), then coarser.  wk/wv and the bulk
            # prefetch (x1-3, wo) go on the scalar queue in parallel (idle
            # in phase 1); x3/wo block that queue until their slot retires,
            # which is long before phase 2 needs ACT.
            xts = [xpool.tile([128, ND * 512], f16, tag="xt", name=f"xt{sc}") for sc in range(NQ)]
            # chunk-0 pieces interleaved across all four tensors at 2-d-tile
            # (later 4-d-tile) granularity: the fat chunk-0 loop below
            # consumes ~200GB/s in d-tile order, matching early DMA rate.
            for pc, w in ((0, 1), (1, 1), (2, 2), (4, 2), (6, 2), (8, 4), (12, 4)):
                nc.sync.dma_start(
                    out=xts[0][:, pc * 512:(pc + w) * 512],
                    in_=xt4[0, :, pc * 512:(pc + w) * 512],
                )
                for wt, wsb in ((wq, wq_sb), (wk, wk_sb), (wv, wv_sb)):
                    nc.sync.dma_start(
                        out=wsb[:, pc * DPC:(pc + w) * DPC],
                        in_=wt[:, pc * DPC:(pc + w) * DPC],
                    )
            # bulk prefetch after the chunk-0 stream (avoids stealing its
            # bandwidth).  Everything stays on the sync queue: x3/wo block
            # their queue until their xt slot retires, and parking them on
            # the scalar queue would stall the phase-1 copies ACT now takes.
            nc.sync.dma_start(out=xts[1][:], in_=xt4[1, :, :])
            nc.sync.dma_start(out=xts[2][:], in_=xt4[2, :, :])
            nc.sync.dma_start(out=xts[3][:], in_=xt4[3, :, :])
            wo_ot = xpool.tile([128, ND * 512], f16, tag="xt", name="wo_ot")
            wo_sb = wo_ot[:, 0:2 * S]
            nc.sync.dma_start(out=wo_sb, in_=wo[:])

            # ---------------- phase 1: projections ----------------
            # chunk 0: DMA-paced interleaved accumulation -- per d-tile all
            # eight output groups advance together (Q/K both heads in the
            # two banks of a [128,1024] psC tile each, V s-tiles packed two
            # per psA bank), so each arriving 2-d-tile DMA piece is consumed
            # once and the PE runs from the first piece instead of waiting
            # for whole tensors.
            qacc = psC.tile([128, 1024], f32, tag="st", name="qacc")
            kacc = psC.tile([128, 1024], f32, tag="st", name="kacc")
            vacc = [psA.tile([128, 512], f32, tag="proj", name=f"vacc{i}")
                    for i in range(2)]
            xt0 = xts[0]
            for dt_ in range(ND):
                for acc_, w_sb in ((qacc, wq_sb), (kacc, wk_sb)):
                    for h in range(2):
                        nc.tensor.matmul(
                            acc_[:, h * 512:(h + 1) * 512],
                            w_sb[:, dt_ * DPC + h * 128: dt_ * DPC + h * 128 + 128],
                            xt0[:, dt_ * 512:(dt_ + 1) * 512],
                            start=(dt_ == 0), stop=(dt_ == ND - 1),
                        )
                for s in range(4):
                    # start=True clears has_written for the WHOLE bank, so
                    # only the first of the two groups sharing a bank may
                    # clear; the second overwrites-where-unset on its first
                    # matmul (its region's bits are clear after the even
                    # group's start).
                    nc.tensor.matmul(
                        vacc[s // 2][:, (s % 2) * 256:(s % 2) * 256 + 256],
                        xt0[:, dt_ * 512 + s * 128: dt_ * 512 + s * 128 + 128],
                        wv_sb[:, dt_ * DPC:(dt_ + 1) * DPC],
                        start=(dt_ == 0 and s % 2 == 0),
                        stop=(dt_ == ND - 1),
                        skip_group_check=True,
                    )
            # copies alternate DVE/ACT (ACT is idle in phase 1) so psA/psC
            # slots free twice as fast at chunk boundaries
            for h in range(2):
                cp0 = nc.vector.tensor_copy if h == 0 else (
                    lambda o, i: nc.scalar.activation(o, i, CPY))
                cp0(qt_sb[h][:, 0:512], qacc[:, h * 512:(h + 1) * 512])
                cp0(kt_sb[h][:, 0:512], kacc[:, h * 512:(h + 1) * 512])
            for s in range(4):
                dst_v = v_sb[:, s * DPC:(s + 1) * DPC]
                src_v = vacc[s // 2][:, (s % 2) * 256:(s % 2) * 256 + 256]
                if s % 2 == 0:
                    nc.vector.tensor_copy(dst_v, src_v)
                else:
                    nc.scalar.activation(dst_v, src_v, CPY)

            for sc in range(1, NQ):  # remaining chunks of 512 seq positions
                xt = xts[sc]
                # chunk 3 computes K^T/V first so attention can start
                # before its Q^T (only needed by the last q-chunk) is done
                wlist = ((wq_sb, qt_sb), (wk_sb, kt_sb))
                if sc == NQ - 1:
                    wlist = ((wk_sb, kt_sb), (wq_sb, qt_sb))

                def emit_v():
                    # V natural: [s_tile 128, 256] = sum_d xT[d, s_tile].T @ Wv[d, :]
                    for st in range(4):
                        s_tile = sc * 4 + st
                        ps = psA.tile([128, 512], f32, tag="proj", name="v_ps")
                        for dt_ in range(ND):
                            nc.tensor.matmul(
                                ps[:, 0:DPC],
                                xt[:, dt_ * 512 + st * 128: dt_ * 512 + st * 128 + 128],
                                wv_sb[:, dt_ * DPC:(dt_ + 1) * DPC],
                                start=(dt_ == 0),
                                stop=(dt_ == ND - 1),
                            )
                        if st % 2 == 0:
                            nc.vector.tensor_copy(
                                v_sb[:, s_tile * DPC:(s_tile + 1) * DPC], ps[:, 0:DPC]
                            )
                        else:
                            nc.scalar.activation(
                                v_sb[:, s_tile * DPC:(s_tile + 1) * DPC], ps[:, 0:DPC], CPY
                            )

                first = True
                for w_sb, dst in wlist:
                    # Q^T/K^T: [n_tile 128, s 512] = sum_d W[d, n].T @ xT[d, s]
                    for h in range(2):
                        ps = psA.tile([128, 512], f32, tag="proj", name="qk_ps")
                        for dt_ in range(ND):
                            nc.tensor.matmul(
                                ps[:],
                                w_sb[:, dt_ * DPC + h * 128: dt_ * DPC + h * 128 + 128],
                                xt[:, dt_ * 512:(dt_ + 1) * 512],
                                start=(dt_ == 0),
                                stop=(dt_ == ND - 1),
                            )
                        if h == 0:
                            nc.vector.tensor_copy(
                                dst[h][:, sc * 512:(sc + 1) * 512], ps[:]
                            )
                        else:
                            nc.scalar.activation(
                                dst[h][:, sc * 512:(sc + 1) * 512], ps[:], CPY
                            )
                    if sc == NQ - 1 and first:
                        emit_v()  # last chunk: V right after K^T
                    first = False
                if sc != NQ - 1:
                    emit_v()

            # ------- phases 2+3 interleaved -------
            # attention per (q-chunk, head); the output projection for
            # q-chunk qc is emitted one group per attention-group during the
            # next two iterations, keeping the PE dense and the tail short.
            ot_tiles = {}
            pending = []  # (qc, qt_local, ec) proj groups ready to emit
            tail_mode = [0]  # >0 once the final flush starts (copy counter)

            tail_ps = [None]

            def emit_proj_group():
                qc_, qt_, ec = pending.pop(0)
                if tail_mode[0]:
                    # final flush: attention is done, so psB is free --
                    # alternate groups between psA and psB so the
                    # write-after-read distance is 4 groups instead of 2
                    # (pool WAR deps are per-tile-slot, and a 2-deep
                    # rotation leaves every copy's semaphore round-trip on
                    # the PE critical path: ~700ns/group stalls).  Copies
                    # alternate strictly ACT/DVE for the same reason.
                    c = tail_mode[0] - 1
                    tail_mode[0] += 1
                    if c % 2 == 0:
                        ps = psA.tile([128, 512], f32, tag="proj", name="tail_psA")[:]
                    else:
                        ps = psB.tile([128, 512], f32, tag="oT", name="tail_psB")[:]
                    on_act = c % 2 == 0
                else:
                    ps = psA.tile([128, 512], f32, tag="proj", name="proj_ps")[:]
                    on_act = ec == 0
                for dt_ in range(2):
                    nc.tensor.matmul(
                        ps,
                        ot_tiles[(dt_, qc_)][:, qt_ * 128:(qt_ + 1) * 128],
                        wo_sb[:, dt_ * S + ec * 512:dt_ * S + ec * 512 + 512],
                        start=(dt_ == 0),
                        stop=(dt_ == 1),
                    )
                ob = ob_tiles[(qc_, qt_)]
                if on_act:
                    nc.scalar.activation(ob[:, ec * 512:(ec + 1) * 512], ps, CPY)
                else:
                    nc.vector.tensor_copy(ob[:, ec * 512:(ec + 1) * 512], ps)
                if ec % 2 == 1:  # DMA each half as soon as its 2 copies land
                    h_ = ec // 2
                    nc.sync.dma_start(
                        out=out[qc_ * 4 + qt_, :, h_ * 1024:(h_ + 1) * 1024],
                        in_=ob[:, h_ * 1024:(h_ + 1) * 1024],
                    )

            ob_tiles = {}
            for qc in range(NQ):
                for h in range(2):
                    oT = psB.tile([128, 512], f32, tag="oT", name="oT")
                    qt_slice = qt_sb[h][:, qc * 512:(qc + 1) * 512]
                    acc = apool.tile([128, 1024], f16, tag="acc", name="acc")
                    st = psC.tile([128, 1024], f32, tag="st", name="st0")
                    for half in range(2):
                        nc.tensor.matmul(
                            st[:, half * 512:(half + 1) * 512],
                            kt_sb[h][:, half * 128:(half + 1) * 128],
                            qt_slice,
                            start=True, stop=True,
                        )
                    for g in range(8):  # 8 groups of 2 k-tiles
                        pt = ppool.tile([128, 1024], f16, tag="pt", name="pt")
                        nc.scalar.activation(pt[:], st[:], EXP, scale=float(SCALE))
                        if g + 1 < 8:
                            st = psC.tile([128, 1024], f32, tag="st", name="stn")
                            for half in range(2):
                                kt_ = (g + 1) * 2 + half
                                nc.tensor.matmul(
                                    st[:, half * 512:(half + 1) * 512],
                                    kt_sb[h][:, kt_ * 128:(kt_ + 1) * 128],
                                    qt_slice,
                                    start=True, stop=True,
                                )
                        if g == 0:
                            nc.vector.tensor_copy(acc[:], pt[:])
                        else:
                            nc.vector.tensor_add(acc[:], acc[:], pt[:])
                        for half in range(2):
                            kt_ = g * 2 + half
                            nc.tensor.matmul(
                                oT[:],
                                v_sb[:, kt_ * DPC + h * 128: kt_ * DPC + h * 128 + 128],
                                pt[:, half * 512:(half + 1) * 512],
                                start=(kt_ == 0), stop=(kt_ == NS - 1),
                            )
                        if pending:
                            emit_proj_group()
                    # softmax denominator broadcast to all partitions via a
                    # ones-lhsT matmul over the f16 accumulator
                    bc = psA.tile([128, 512], f32, tag="proj", name="bc")
                    nc.tensor.matmul(bc[:], ones_sb[:], acc[:, 0:512],
                                     start=True, stop=False)
                    nc.tensor.matmul(bc[:], ones_sb[:], acc[:, 512:1024],
                                     start=False, stop=True)
                    rc = rpool.tile([128, 512], f32, tag="recip", name="rc")
                    # ~18-bit 1/x, ~5x faster than exact reciprocal; softmax
                    # denominators are well-conditioned positives (~1e2..4e3)
                    nc.vector.reciprocal_approx_fast(rc[:], bc[:])
                    ot_tiles[(h, qc)] = opool.tile(
                        [128, 512], f16, tag="ot", name="ot_t"
                    )
                    nc.vector.tensor_mul(ot_tiles[(h, qc)][:], oT[:], rc[:])
                # both heads of qc done: queue its 16 proj groups (qt-major
                # so each out row's 4 copies finish before its DMA)
                for qt_ in range(4):
                    ob_tiles[(qc, qt_)] = obpool.tile(
                        [128, S], f16, tag="ob", name="ob"
                    )
                    for ec in range(NQ):
                        pending.append((qc, qt_, ec))
            tail_mode[0] = 1
            while pending:
                emit_proj_group()

    nc.compile()
    return nc


def _numpy_fallback(x, mask, Wq, bq, Wk, bk, Wv, bv, Wo, bo):
    B, S_, D_ = x.shape
    xf = x.reshape(S_, D_).astype(np.float64)

    def proj(W, b):
        y = xf @ W.astype(np.float64) + b.astype(np.float64)
        return y.reshape(S_, H, DK).transpose(1, 0, 2)

    Q = proj(Wq, bq)
    K = proj(Wk, bk)
    V = proj(Wv, bv)
    m = np.broadcast_to(mask, (B, H, S_, S_))
    out = np.empty((H, S_, DK))
    for h in range(H):
        sc = Q[h] @ K[h].T / np.sqrt(DK)
        sc = np.where(m[0, h], sc, -np.inf)
        sc -= sc.max(axis=-1, keepdims=True)
        e = np.exp(sc)
        p = e / e.sum(axis=-1, keepdims=True)
        out[h] = p @ V[h]
    o = out.transpose(1, 0, 2).reshape(S_, D_)
    res = o @ Wo.astype(np.float64) + bo.astype(np.float64)
    return res.reshape(B, S_, D_).astype(np.float32)


def kernel(x, mask, Wq, bq, Wk, bk, Wv, bv, Wo, bo):
    x = np.asarray(x, dtype=np.float32)
    mask = np.asarray(mask)
    Wq = np.asarray(Wq, dtype=np.float32)
    Wk = np.asarray(Wk, dtype=np.float32)
    Wv = np.asarray(Wv, dtype=np.float32)
    Wo = np.asarray(Wo, dtype=np.float32)
    bq = np.asarray(bq, dtype=np.float32)
    bk = np.asarray(bk, dtype=np.float32)
    bv = np.asarray(bv, dtype=np.float32)
    bo = np.asarray(bo, dtype=np.float32)

    # Off-benchmark shapes/masks/biases: exact numpy fallback.
    # (bk shifts every score row by a constant -> softmax-invariant; bv and bo
    # are affine in the output and folded in on the host; only bq actually
    # changes the attention pattern in a way the device kernel doesn't model.)
    if x.shape != (1, S, D) or not bool(mask.all()) or np.any(bq):
        return _numpy_fallback(x, mask, Wq, bq, Wk, bk, Wv, bv, Wo, bo)

    from concourse.bass_utils import run_bass_kernel_spmd

    if _PROGRAM[0] is None:
        _PROGRAM[0] = _build_program()
    nc = _PROGRAM[0]

    # xT[dt*128+p, sc*512+s] -> [sc, p, dt, s] (16KB contiguous per partition)
    xT = x.reshape(S, D).T.astype(np.float16)
    xt4 = np.ascontiguousarray(
        xT.reshape(16, 128, 4, 512).transpose(2, 1, 0, 3)
    ).reshape(4, 128, 16 * 512)

    def wlay(Wcol):  # [dt*128+p, n] -> [p, dt*n]
        return np.ascontiguousarray(
            Wcol.reshape(16, 128, DPC).transpose(1, 0, 2)
        ).reshape(128, 16 * DPC)

    in_maps = []
    for c in range(N_CORES):
        lo, hi = c * DPC, (c + 1) * DPC
        wo_c = np.ascontiguousarray(
            Wo[lo:hi, :].astype(np.float16).reshape(2, 128, S).transpose(1, 0, 2)
        ).reshape(128, 2 * S)
        in_maps.append(
            {
                "xt4": xt4,
                "wq": wlay(Wq[:, lo:hi].astype(np.float16)),
                "wk": wlay(Wk[:, lo:hi].astype(np.float16)),
                "wv": wlay(Wv[:, lo:hi].astype(np.float16)),
                "wo": wo_c,
            }
        )

    res = run_bass_kernel_spmd(nc, in_maps, list(range(N_CORES)), trace=TRACE)
    _LAST_EXEC_NS[0] = res.exec_time_ns
    _LAST_RESULTS[0] = res

    acc = res.results[0]["out"].astype(np.float64)
    for c in range(1, N_CORES):
        acc += res.results[c]["out"]
    acc = acc.reshape(S, D)
    # bv contributes (attn rows sum to 1) a constant bv @ Wo; bo is additive.
    acc += (bv.astype(np.float64) @ Wo) + bo
    return acc.astype(np.float32).reshape(1, S, D)
